# revision 5
# baseline (speedup 1.0000x reference)
"""Causal MHA (B=1, S=4096, D=1024, H=16, hd=64) on 8 trn2 cores.

v2: head-parallel (2 heads/core) like the baseline, plus:
  - QKV projections in fp8e4 with DoubleRow (k=256 per pass), weights
    pre-scaled by 32 on host to stay in fp8 normal range; the 32*32
    score scale is folded into the softmax exp scale, and the 32x v
    scale cancels via a ones-column of 32 in v_sb.
  - V computed token-major directly (no PE transposes).
  - exp probabilities written as fp8e4; context matmul in fp8 DoubleRow
    over key-tile pairs (contraction 256/pass).
  - partial out written bf16 (half the DMA), summed in f32 on host.
"""

import sys

import numpy as np

for _p in ("/opt/trn_rl_repo", "/root/.axon_site/_ro/trn_rl_repo"):
    if _p not in sys.path:
        sys.path.insert(0, _p)

import concourse.bass as bass  # noqa: E402
import concourse.tile as tile  # noqa: E402
from concourse import bacc, mybir  # noqa: E402
from concourse.bass_utils import run_bass_kernel_spmd  # noqa: E402

F32 = mybir.dt.float32
F32R = mybir.dt.float32r
BF16 = mybir.dt.bfloat16
FP8 = mybir.dt.float8e4
DR = mybir.MatmulPerfMode.DoubleRow
EXP = mybir.ActivationFunctionType.Exp

N_CORES = 8
FPC = 128   # features per core (2 heads x 64)
HD = 64
EXP_SCALE = 0.125


def build_nc(S=4096, D=1024, repeat=1, no_out_dma=False,
             no_exp=False, no_mask=False,
             mm_scores=True, mm_ctx=True, mm_qkv=True, mm_outproj=True):
    NDP = D // 256          # d-chunk PAIRS for DR (4)
    NKT = S // 128          # key tiles (32)
    NPAIR = NKT // 2        # key tile pairs (16)
    NQC = S // 512          # query chunks (8)
    NJ = S // 512           # 512-token j-blocks (8)

    nc = bacc.Bacc("TRN2", target_bir_lowering=False, debug=False,
                   num_devices=N_CORES)

    NDC = D // 128          # d chunks (8)
    # xb: [128, NDC, S] bf16, d = c*128 + p
    xb = nc.dram_tensor("xb", [128, NDC * S], BF16, kind="ExternalInput")
    # w*b: [128, NDC, 128] bf16, d = c*128 + p
    wq = nc.dram_tensor("wqb", [128, NDC * FPC], BF16, kind="ExternalInput")
    wk = nc.dram_tensor("wkb", [128, NDC * FPC], BF16, kind="ExternalInput")
    wv = nc.dram_tensor("wvb", [128, NDC * FPC], BF16, kind="ExternalInput")
    wo = nc.dram_tensor("wo", [FPC, D], F32, kind="ExternalInput")
    out = nc.dram_tensor("out", [S, D], BF16, kind="ExternalOutput")

    with tile.TileContext(nc) as tc:
        with tc.tile_pool(name="const", bufs=1) as const, \
             tc.tile_pool(name="persist", bufs=1) as persist, \
             tc.tile_pool(name="xpool", bufs=2) as xpool, \
             tc.tile_pool(name="eppool", bufs=4) as eppool, \
             tc.tile_pool(name="smalls", bufs=4) as smalls, \
             tc.tile_pool(name="outsb", bufs=5) as outsb, \
             tc.tile_pool(name="qkvps", bufs=2, space="PSUM") as qkvps, \
             tc.tile_pool(name="scps", bufs=2, space="PSUM") as scps, \
             tc.tile_pool(name="ctxps", bufs=1, space="PSUM") as ctxps:
            # constants (loaded once; amortized across repeats)
            wq_sb = const.tile([128, NDC * FPC], BF16, tag="wq")
            wk_sb = const.tile([128, NDC * FPC], BF16, tag="wk")
            wv_sb = const.tile([128, NDC * FPC], BF16, tag="wv")
            wo_sb = const.tile([FPC, D], F32R, tag="wo")
            nc.scalar.dma_start(out=wq_sb[:], in_=wq[:])
            nc.scalar.dma_start(out=wk_sb[:], in_=wk[:])
            nc.scalar.dma_start(out=wv_sb[:], in_=wv[:])
            nc.scalar.dma_start(out=wo_sb[:], in_=wo[:].bitcast(F32R))

            def wch(w_sb, c):
                return w_sb[:, c * FPC:(c + 1) * FPC]

            # persistent intermediates
            qT = persist.tile([128, S], BF16, tag="qT")    # [feat, tok]
            kT = persist.tile([128, S], BF16, tag="kT")
            # v_sb: [128 keys, (pair*2+h)*160 + s*80 + (0:64|ones|pad)] fp8
            # 80 per sub: DoubleRow ldweights requires the per-sub
            # stationary width to be a multiple of 16.
            v_sb = persist.tile([128, NPAIR * 2 * 160], FP8, tag="v_sb")
            v_lo = persist.tile([128, NPAIR * 2 * 160], FP8, tag="v_lo")
            ctxT = persist.tile([128, S], F32R, tag="ctxT")

            # ones columns; pad cols zeroed. v_lo's ones column is 0 so
            # the softmax denominator is only counted once.
            nc.vector.memset(v_sb[:], 0.0)
            nc.vector.memset(v_lo[:], 0.0)
            for _ph in range(NPAIR * 2):
                for _s in range(2):
                    nc.vector.memset(
                        v_sb[:, _ph * 160 + _s * 80 + 64:
                                _ph * 160 + _s * 80 + 65], 1.0)

            def vdr(pair, h, t=None):
                base = (pair * 2 + h) * 160
                tt = v_sb if t is None else t
                return tt[:, base:base + 160].rearrange(
                    "p (s m) -> p s m", s=2)

            for _rep in range(repeat):

                def emit_j(j, after_j=None):
                    xt = xpool.tile([128, NDC * 512], BF16, tag="x",
                                    name="x")
                    xv = xb[:].rearrange("p (c t) -> p c t",
                                         c=NDC)[:, :, j * 512:(j + 1) * 512]
                    nc.sync.dma_start(
                        out=xt[:].rearrange("p (c t) -> p c t", c=NDC),
                        in_=xv)
                    xr = xt[:].rearrange("p (c t) -> p c t", c=NDC)
                    ndc_eff = NDC if mm_qkv else NDC // 2
                    for (w_sb, kind) in ((wq_sb, "q"), (wk_sb, "k")):
                        ps = qkvps.tile([128, 512], F32, tag="qkv",
                                        name="ps")
                        for c in range(ndc_eff):
                            nc.tensor.matmul(
                                ps[:], wch(w_sb, c), xr[:, c],
                                start=(c == 0), stop=(c == ndc_eff - 1))
                        col = j * 512
                        dst = qT if kind == "q" else kT
                        nc.vector.tensor_copy(dst[:, col:col + 512], ps[:])
                    # V token-major: out [128 tok, 128 feat] per tok tile
                    pv = qkvps.tile([128, 512], F32, tag="qkv", name="pv")
                    for t in range(4):
                        for c in range(ndc_eff):
                            nc.tensor.matmul(
                                pv[:, t * 128:(t + 1) * 128],
                                xr[:, c, t * 128:(t + 1) * 128],
                                wch(wv_sb, c),
                                start=(c == 0), stop=(c == ndc_eff - 1),
                                skip_group_check=True)
                        kt = j * 4 + t
                        pair, s = kt // 2, kt & 1
                        # both heads in one strided copy:
                        # dst [128, h, 64], src psum [128, h, 64]
                        dst = v_sb[:, pair * 320:pair * 320 + 320] \
                            .rearrange("p (h s m) -> p h s m",
                                       h=2, s=2)[:, :, s, 0:64]
                        dst_lo = v_lo[:, pair * 320:pair * 320 + 320] \
                            .rearrange("p (h s m) -> p h s m",
                                       h=2, s=2)[:, :, s, 0:64]
                        src = pv[:, t * 128:(t + 1) * 128].rearrange(
                            "p (h m) -> p h m", h=2)
                        nc.vector.tensor_copy(dst, src)
                        nc.vector.tensor_sub(dst_lo, src, dst)
                    if after_j is not None:
                        after_j((j + 1) * 512)

                state = {"prev": None}

                def emit_norm(qc_, ctx_):
                    for h in range(2):
                        rrow = smalls.tile([1, 512], F32, tag="rrow",
                                           name="rrow")
                        nc.vector.reciprocal(rrow[:], ctx_[h][64:65, :])
                        rb = smalls.tile([64, 512], F32, tag="rb", name="rb")
                        nc.gpsimd.partition_broadcast(rb[:], rrow[:])
                        nc.vector.tensor_mul(
                            ctxT[64 * h:64 * h + 64,
                                 qc_ * 512:(qc_ + 1) * 512],
                            ctx_[h][0:64, :].bitcast(F32R),
                            rb[:].bitcast(F32R))

                def emit_outproj(qc_, ts_=(0, 1, 2, 3)):
                    for t in ts_:
                        qt = qc_ * 4 + t
                        ot = outsb.tile([128, D], BF16, tag="ot", name="ot")
                        for g in range(D // 512 if mm_outproj else 1):
                            po = qkvps.tile([128, 512], F32, tag="qkv",
                                            name="po")
                            nc.tensor.matmul(
                                po[:], ctxT[:, qt * 128:(qt + 1) * 128],
                                wo_sb[:, g * 512:(g + 1) * 512],
                                start=True, stop=True)
                            nc.vector.tensor_copy(
                                ot[:, g * 512:(g + 1) * 512], po[:])
                        if not no_out_dma:
                            nc.scalar.dma_start(
                                out=out[qt * 128:(qt + 1) * 128,
                                        :D if mm_outproj else 512],
                                in_=ot[:, :D if mm_outproj else 512])

                def emit_chunk(qc):
                    kmax = 4 * qc + 4
                    pmax = kmax // 2
                    ctx = []
                    for h in range(2):
                        cx = ctxps.tile([80, 512], F32, tag=f"ctx{h}",
                                        name=f"ctx{h}")
                        ctx.append(cx)
                    pend = []

                    def emit_ctx(args):
                        pair_, eps_ = args
                        for h in range(2):
                            if h == 1 and not mm_ctx and 0 < pair_ < pmax - 1:
                                continue
                            epr = eps_[h][:].rearrange("p (s m) -> p s m",
                                                       s=2)
                            nc.tensor.matmul(
                                ctx[h][:, :], vdr(pair_, h), epr,
                                start=(pair_ == 0), stop=False,
                                perf_mode=DR, skip_group_check=True)
                            nc.tensor.matmul(
                                ctx[h][:, :], vdr(pair_, h, v_lo), epr,
                                start=False, stop=(pair_ == pmax - 1),
                                perf_mode=DR, skip_group_check=True)

                    for pair in range(pmax):
                        eps = []
                        diag = (pair >= 2 * qc)
                        for h in range(2):
                            ep = eppool.tile([128, 1024], FP8, tag="ep",
                                             name="ep")
                            if diag:
                                nc.vector.memset(ep[:], 0.0)
                            eps.append(ep)
                        for si in range(2):
                            kt = pair * 2 + si
                            s0 = max(0, kt * 128 - qc * 512)
                            sc = scps.tile([128, 1024], F32, tag="sc",
                                           name="sc")
                            for h in range(2 if (mm_scores or kt < 2) else 1):
                                nc.tensor.matmul(
                                    sc[:, h * 512 + s0:h * 512 + 512],
                                    kT[64 * h:64 * h + 64,
                                       kt * 128:(kt + 1) * 128],
                                    qT[64 * h:64 * h + 64,
                                       qc * 512 + s0:(qc + 1) * 512],
                                    start=True, stop=True)
                            if not no_exp:
                                for h in range(2):
                                    nc.scalar.activation(
                                        eps[h][:, si * 512 + s0:
                                               si * 512 + 512],
                                        sc[:, h * 512 + s0:h * 512 + 512],
                                        EXP, scale=EXP_SCALE)
                            if kt >= 4 * qc and not no_mask:
                                j0 = kt * 128 - qc * 512
                                for h in range(2):
                                    blk = eps[h][:, si * 512 + j0:
                                                 si * 512 + j0 + 128]
                                    nc.gpsimd.affine_select(
                                        out=blk, in_=blk,
                                        compare_op=mybir.AluOpType.is_ge,
                                        fill=0.0, base=0,
                                        pattern=[[1, 128]],
                                        channel_multiplier=-1)
                        pend.append((pair, eps))
                        if len(pend) > 1:
                            emit_ctx(pend.pop(0))
                        if pair == 0 and state["prev"] is not None:
                            emit_norm(*state["prev"])
                        if state["prev"] is not None and 1 <= pair < 3 \
                                and pmax > 3:
                            emit_outproj(state["prev"][0],
                                         (2 * (pair - 1), 2 * pair - 1))
                        elif pair == 1 and pmax <= 3 \
                                and state["prev"] is not None:
                            emit_outproj(state["prev"][0])
                    while pend:
                        emit_ctx(pend.pop(0))
                    state["prev"] = (qc, ctx)

                prog = {"chunks": 0}

                def after_j(tokens_done):
                    while (prog["chunks"] + 1) * 512 <= tokens_done \
                            and prog["chunks"] < NQC:
                        emit_chunk(prog["chunks"])
                        prog["chunks"] += 1

                for j in range(NJ):
                    emit_j(j, after_j=after_j)
                while prog["chunks"] < NQC:
                    emit_chunk(prog["chunks"])
                    prog["chunks"] += 1
                emit_norm(*state["prev"])
                emit_outproj(state["prev"][0])

    nc.compile()
    return nc


_NC_CACHE = {}


def _get_nc(S, D):
    key = (S, D)
    if key not in _NC_CACHE:
        _NC_CACHE[key] = build_nc(S, D)
    return _NC_CACHE[key]


def make_in_maps(x, Wq, Wk, Wv, Wo, S, D):
    import ml_dtypes
    BF16NP = ml_dtypes.bfloat16
    NDC = D // 128
    x_f = np.asarray(x, dtype=np.float32).reshape(S, D)
    # xb [128, NDC, S]: d = c*128 + p
    xb = np.ascontiguousarray(
        x_f.T.reshape(NDC, 128, S).transpose(1, 0, 2)
    ).astype(BF16NP).reshape(128, NDC * S)

    in_maps = []
    for c in range(N_CORES):
        sl = slice(c * FPC, (c + 1) * FPC)

        def prepb(W):
            # [D, 128] -> [128p, NDC, 128feat] with d = c*128+p
            Wc = np.asarray(W[:, sl], dtype=np.float32)
            return np.ascontiguousarray(
                Wc.reshape(NDC, 128, FPC).transpose(1, 0, 2)
            ).astype(BF16NP).reshape(128, NDC * FPC)

        in_maps.append({
            "xb": xb,
            "wqb": prepb(Wq),
            "wkb": prepb(Wk),
            "wvb": prepb(Wv),
            "wo": np.ascontiguousarray(np.asarray(Wo[sl, :],
                                                  dtype=np.float32)),
        })
    return in_maps


def kernel(x, Wq, Wk, Wv, Wo, bo):
    x = np.asarray(x, dtype=np.float32)
    B, S, D = x.shape
    nc = _get_nc(S, D)
    in_maps = make_in_maps(x, Wq, Wk, Wv, Wo, S, D)
    res = run_bass_kernel_spmd(nc, in_maps, core_ids=list(range(N_CORES)))
    acc = np.zeros((S, D), dtype=np.float32)
    for c in range(N_CORES):
        acc += np.asarray(res.results[c]["out"], dtype=np.float32)
    acc += np.asarray(bo, dtype=np.float32)[None, :]
    return acc.reshape(B, S, D)


# revision 7
# speedup vs baseline: 1.4501x; 1.4501x over previous
"""Causal MHA (B=1, S=4096, D=1024, H=16, hd=64) on 8 trn2 cores.

Head-parallel (2 heads/core, tensor parallel), row-sharded Wo,
host-side partial-sum reduce. Relative to the f32 baseline:
  - x streamed as bf16 (half the input DMA); QKV projections in bf16.
  - V computed token-major directly (no PE transposes), stored as an
    fp8e4 hi+lo residual pair so the context matmul can use fp8
    DoubleRow (256-deep contraction per pass) without fp8's
    quantization error on V.
  - exp probabilities written as fp8e4 by the activation engine;
    softmax denominator accumulated via a ones-column in the
    DoubleRow stationary (v_lo's ones column is zero).
  - scores keep both heads' queries in separate qT column slabs with
    dead partitions zeroed, so both heads share the same 128-deep
    kT stationary per key tile.
  - causal mask via a single widened affine_select per diagonal tile
    (also zeroes the exp-unwritten prefix).
  - partial out written bf16 (half the output DMA), summed f32 on host.
"""

import sys

import numpy as np

for _p in ("/opt/trn_rl_repo", "/root/.axon_site/_ro/trn_rl_repo"):
    if _p not in sys.path:
        sys.path.insert(0, _p)

import concourse.bass as bass  # noqa: E402
import concourse.tile as tile  # noqa: E402
from concourse import bacc, mybir  # noqa: E402
from concourse.bass_utils import run_bass_kernel_spmd  # noqa: E402

F32 = mybir.dt.float32
F32R = mybir.dt.float32r
BF16 = mybir.dt.bfloat16
FP8 = mybir.dt.float8e4
DR = mybir.MatmulPerfMode.DoubleRow
EXP = mybir.ActivationFunctionType.Exp

N_CORES = 8
FPC = 128   # features per core (2 heads x 64)
HD = 64
EXP_SCALE = 0.125


def build_nc(S=4096, D=1024, repeat=1, no_out_dma=False,
             no_exp=False, no_mask=False,
             mm_scores=True, mm_ctx=True, mm_qkv=True, mm_outproj=True):
    NDP = D // 256          # d-chunk PAIRS for DR (4)
    NKT = S // 128          # key tiles (32)
    NPAIR = NKT // 2        # key tile pairs (16)
    NQC = S // 512          # query chunks (8)
    NJ = S // 512           # 512-token j-blocks (8)

    nc = bacc.Bacc("TRN2", target_bir_lowering=False, debug=False,
                   num_devices=N_CORES)

    NDC = D // 128          # d chunks (8)
    # xb: [128, NDC, S] bf16, d = c*128 + p
    xb = nc.dram_tensor("xb", [128, NDC * S], BF16, kind="ExternalInput")
    # w*b: [128, NDC, 128] bf16, d = c*128 + p
    wq = nc.dram_tensor("wqb", [128, NDC * FPC], BF16, kind="ExternalInput")
    wk = nc.dram_tensor("wkb", [128, NDC * FPC], BF16, kind="ExternalInput")
    wv = nc.dram_tensor("wvb", [128, NDC * FPC], BF16, kind="ExternalInput")
    wo = nc.dram_tensor("wo", [FPC, D], F32, kind="ExternalInput")
    out = nc.dram_tensor("out", [S, D], BF16, kind="ExternalOutput")

    with tile.TileContext(nc) as tc:
        with tc.tile_pool(name="const", bufs=1) as const, \
             tc.tile_pool(name="persist", bufs=1) as persist, \
             tc.tile_pool(name="xpool", bufs=2) as xpool, \
             tc.tile_pool(name="eppool", bufs=4) as eppool, \
             tc.tile_pool(name="smalls", bufs=4) as smalls, \
             tc.tile_pool(name="outsb", bufs=5) as outsb, \
             tc.tile_pool(name="qkvps", bufs=2, space="PSUM") as qkvps, \
             tc.tile_pool(name="scps", bufs=2, space="PSUM") as scps, \
             tc.tile_pool(name="ctxps", bufs=1, space="PSUM") as ctxps:
            # constants (loaded once; amortized across repeats)
            wq_sb = const.tile([128, NDC * FPC], BF16, tag="wq")
            wk_sb = const.tile([128, NDC * FPC], BF16, tag="wk")
            wv_sb = const.tile([128, NDC * FPC], BF16, tag="wv")
            wo_sb = const.tile([FPC, D], F32R, tag="wo")
            nc.scalar.dma_start(out=wq_sb[:], in_=wq[:])
            nc.scalar.dma_start(out=wk_sb[:], in_=wk[:])
            nc.scalar.dma_start(out=wv_sb[:], in_=wv[:])
            nc.scalar.dma_start(out=wo_sb[:], in_=wo[:].bitcast(F32R))

            def wch(w_sb, c):
                return w_sb[:, c * FPC:(c + 1) * FPC]

            # persistent intermediates
            # qT: [128, h, S] — head h's queries live on that head's 64
            # partitions; the other 64 partitions are zero. This lets the
            # score matmuls use the full-128 kT block as stationary, shared
            # by both heads (second ldweights elided).
            qT = persist.tile([128, 2 * S], BF16, tag="qT")
            kT = persist.tile([128, S], BF16, tag="kT")
            nc.vector.memset(qT[:], 0.0)
            # v_sb: [128 keys, (pair*2+h)*160 + s*80 + (0:64|ones|pad)] fp8
            # 80 per sub: DoubleRow ldweights requires the per-sub
            # stationary width to be a multiple of 16.
            v_sb = persist.tile([128, NPAIR * 2 * 160], FP8, tag="v_sb")
            v_lo = persist.tile([128, NPAIR * 2 * 160], FP8, tag="v_lo")
            ctxT = persist.tile([128, S], F32R, tag="ctxT")

            # ones columns; pad cols zeroed. v_lo's ones column is 0 so
            # the softmax denominator is only counted once.
            nc.vector.memset(v_sb[:], 0.0)
            nc.vector.memset(v_lo[:], 0.0)
            for _ph in range(NPAIR * 2):
                for _s in range(2):
                    nc.vector.memset(
                        v_sb[:, _ph * 160 + _s * 80 + 64:
                                _ph * 160 + _s * 80 + 65], 1.0)

            def vdr(pair, h, t=None):
                base = (pair * 2 + h) * 160
                tt = v_sb if t is None else t
                return tt[:, base:base + 160].rearrange(
                    "p (s m) -> p s m", s=2)

            for _rep in range(repeat):

                def emit_j(j, after_j=None):
                    xt = xpool.tile([128, NDC * 512], BF16, tag="x",
                                    name="x")
                    xv = xb[:].rearrange("p (c t) -> p c t",
                                         c=NDC)[:, :, j * 512:(j + 1) * 512]
                    nc.sync.dma_start(
                        out=xt[:].rearrange("p (c t) -> p c t", c=NDC),
                        in_=xv)
                    xr = xt[:].rearrange("p (c t) -> p c t", c=NDC)
                    ndc_eff = NDC if mm_qkv else NDC // 2
                    for (w_sb, kind) in ((wq_sb, "q"), (wk_sb, "k")):
                        ps = qkvps.tile([128, 512], F32, tag="qkv",
                                        name="ps")
                        for c in range(ndc_eff):
                            nc.tensor.matmul(
                                ps[:], wch(w_sb, c), xr[:, c],
                                start=(c == 0), stop=(c == ndc_eff - 1))
                        col = j * 512
                        if kind == "q":
                            qr = qT[:].rearrange("p (h t) -> p h t", h=2)
                            nc.vector.tensor_copy(
                                qr[0:64, 0, col:col + 512], ps[0:64, :])
                            nc.vector.tensor_copy(
                                qr[64:128, 1, col:col + 512], ps[64:128, :])
                        else:
                            nc.vector.tensor_copy(kT[:, col:col + 512],
                                                  ps[:])
                    # V token-major: out [128 tok, 128 feat] per tok tile
                    pv = qkvps.tile([128, 512], F32, tag="qkv", name="pv")
                    for t in range(4):
                        for c in range(ndc_eff):
                            nc.tensor.matmul(
                                pv[:, t * 128:(t + 1) * 128],
                                xr[:, c, t * 128:(t + 1) * 128],
                                wch(wv_sb, c),
                                start=(c == 0), stop=(c == ndc_eff - 1),
                                skip_group_check=True)
                        kt = j * 4 + t
                        pair, s = kt // 2, kt & 1
                        # both heads in one strided copy:
                        # dst [128, h, 64], src psum [128, h, 64]
                        dst = v_sb[:, pair * 320:pair * 320 + 320] \
                            .rearrange("p (h s m) -> p h s m",
                                       h=2, s=2)[:, :, s, 0:64]
                        dst_lo = v_lo[:, pair * 320:pair * 320 + 320] \
                            .rearrange("p (h s m) -> p h s m",
                                       h=2, s=2)[:, :, s, 0:64]
                        src = pv[:, t * 128:(t + 1) * 128].rearrange(
                            "p (h m) -> p h m", h=2)
                        nc.vector.tensor_copy(dst, src)
                        nc.vector.tensor_sub(dst_lo, src, dst)
                    if after_j is not None:
                        after_j((j + 1) * 512)

                state = {"prev": None}

                def emit_norm(qc_, ctx_):
                    for h in range(2):
                        rrow = smalls.tile([1, 512], F32, tag="rrow",
                                           name="rrow")
                        nc.vector.reciprocal(rrow[:], ctx_[h][64:65, :])
                        rb = smalls.tile([64, 512], F32, tag="rb", name="rb")
                        nc.gpsimd.partition_broadcast(rb[:], rrow[:])
                        nc.vector.tensor_mul(
                            ctxT[64 * h:64 * h + 64,
                                 qc_ * 512:(qc_ + 1) * 512],
                            ctx_[h][0:64, :].bitcast(F32R),
                            rb[:].bitcast(F32R))

                def emit_outproj(qc_, ts_=(0, 1, 2, 3)):
                    for t in ts_:
                        qt = qc_ * 4 + t
                        ot = outsb.tile([128, D], BF16, tag="ot", name="ot")
                        for g in range(D // 512 if mm_outproj else 1):
                            po = qkvps.tile([128, 512], F32, tag="qkv",
                                            name="po")
                            nc.tensor.matmul(
                                po[:], ctxT[:, qt * 128:(qt + 1) * 128],
                                wo_sb[:, g * 512:(g + 1) * 512],
                                start=True, stop=True)
                            nc.vector.tensor_copy(
                                ot[:, g * 512:(g + 1) * 512], po[:])
                        if not no_out_dma:
                            nc.scalar.dma_start(
                                out=out[qt * 128:(qt + 1) * 128,
                                        :D if mm_outproj else 512],
                                in_=ot[:, :D if mm_outproj else 512])

                def emit_chunk(qc):
                    kmax = 4 * qc + 4
                    pmax = kmax // 2
                    ctx = []
                    for h in range(2):
                        cx = ctxps.tile([80, 512], F32, tag=f"ctx{h}",
                                        name=f"ctx{h}")
                        ctx.append(cx)
                    pend = []

                    def emit_ctx(args):
                        pair_, eps_ = args
                        for h in range(2):
                            if h == 1 and not mm_ctx and 0 < pair_ < pmax - 1:
                                continue
                            epr = eps_[h][:].rearrange("p (s m) -> p s m",
                                                       s=2)
                            nc.tensor.matmul(
                                ctx[h][:, :], vdr(pair_, h), epr,
                                start=(pair_ == 0), stop=False,
                                perf_mode=DR, skip_group_check=True)
                            nc.tensor.matmul(
                                ctx[h][:, :], vdr(pair_, h, v_lo), epr,
                                start=False, stop=(pair_ == pmax - 1),
                                perf_mode=DR, skip_group_check=True)

                    for pair in range(pmax):
                        eps = []
                        diag = (pair >= 2 * qc)
                        for h in range(2):
                            ep = eppool.tile([128, 1024], FP8, tag="ep",
                                             name="ep")
                            eps.append(ep)
                        for si in range(2):
                            kt = pair * 2 + si
                            s0 = max(0, kt * 128 - qc * 512)
                            sc = scps.tile([128, 1024], F32, tag="sc",
                                           name="sc")
                            qr = qT[:].rearrange("p (h t) -> p h t", h=2)
                            for h in range(2 if (mm_scores or kt < 2) else 1):
                                nc.tensor.matmul(
                                    sc[:, h * 512 + s0:h * 512 + 512],
                                    kT[:, kt * 128:(kt + 1) * 128],
                                    qr[:, h,
                                       qc * 512 + s0:(qc + 1) * 512],
                                    start=True, stop=True)
                            if not no_exp:
                                for h in range(2):
                                    nc.scalar.activation(
                                        eps[h][:, si * 512 + s0:
                                               si * 512 + 512],
                                        sc[:, h * 512 + s0:h * 512 + 512],
                                        EXP, scale=EXP_SCALE)
                            if kt >= 4 * qc and not no_mask:
                                # keep col >= row + j0; also zeroes the
                                # exp-unwritten prefix [0, j0)
                                j0 = kt * 128 - qc * 512
                                for h in range(2):
                                    blk = eps[h][:, si * 512:
                                                 si * 512 + j0 + 128]
                                    nc.gpsimd.affine_select(
                                        out=blk, in_=blk,
                                        compare_op=mybir.AluOpType.is_ge,
                                        fill=0.0, base=-j0,
                                        pattern=[[1, j0 + 128]],
                                        channel_multiplier=-1)
                        pend.append((pair, eps))
                        if len(pend) > 1:
                            emit_ctx(pend.pop(0))
                        if pair == 0 and state["prev"] is not None:
                            emit_norm(*state["prev"])
                        if state["prev"] is not None and 1 <= pair < 3 \
                                and pmax > 3:
                            emit_outproj(state["prev"][0],
                                         (2 * (pair - 1), 2 * pair - 1))
                        elif pair == 1 and pmax <= 3 \
                                and state["prev"] is not None:
                            emit_outproj(state["prev"][0])
                    while pend:
                        emit_ctx(pend.pop(0))
                    state["prev"] = (qc, ctx)

                prog = {"chunks": 0}

                def after_j(tokens_done):
                    while (prog["chunks"] + 1) * 512 <= tokens_done \
                            and prog["chunks"] < NQC:
                        emit_chunk(prog["chunks"])
                        prog["chunks"] += 1

                for j in range(NJ):
                    emit_j(j, after_j=after_j)
                while prog["chunks"] < NQC:
                    emit_chunk(prog["chunks"])
                    prog["chunks"] += 1
                emit_norm(*state["prev"])
                emit_outproj(state["prev"][0])

    nc.compile()
    return nc


_NC_CACHE = {}


def _get_nc(S, D):
    key = (S, D)
    if key not in _NC_CACHE:
        _NC_CACHE[key] = build_nc(S, D)
    return _NC_CACHE[key]


def make_in_maps(x, Wq, Wk, Wv, Wo, S, D):
    import ml_dtypes
    BF16NP = ml_dtypes.bfloat16
    NDC = D // 128
    x_f = np.asarray(x, dtype=np.float32).reshape(S, D)
    # xb [128, NDC, S]: d = c*128 + p
    xb = np.ascontiguousarray(
        x_f.T.reshape(NDC, 128, S).transpose(1, 0, 2)
    ).astype(BF16NP).reshape(128, NDC * S)

    in_maps = []
    for c in range(N_CORES):
        sl = slice(c * FPC, (c + 1) * FPC)

        def prepb(W):
            # [D, 128] -> [128p, NDC, 128feat] with d = c*128+p
            Wc = np.asarray(W[:, sl], dtype=np.float32)
            return np.ascontiguousarray(
                Wc.reshape(NDC, 128, FPC).transpose(1, 0, 2)
            ).astype(BF16NP).reshape(128, NDC * FPC)

        in_maps.append({
            "xb": xb,
            "wqb": prepb(Wq),
            "wkb": prepb(Wk),
            "wvb": prepb(Wv),
            "wo": np.ascontiguousarray(np.asarray(Wo[sl, :],
                                                  dtype=np.float32)),
        })
    return in_maps


def kernel(x, Wq, Wk, Wv, Wo, bo):
    x = np.asarray(x, dtype=np.float32)
    B, S, D = x.shape
    nc = _get_nc(S, D)
    in_maps = make_in_maps(x, Wq, Wk, Wv, Wo, S, D)
    res = run_bass_kernel_spmd(nc, in_maps, core_ids=list(range(N_CORES)))
    acc = np.zeros((S, D), dtype=np.float32)
    for c in range(N_CORES):
        acc += np.asarray(res.results[c]["out"], dtype=np.float32)
    acc += np.asarray(bo, dtype=np.float32)[None, :]
    return acc.reshape(B, S, D)
